# revision 2
# baseline (speedup 1.0000x reference)
"""Trainium2 Bass kernel v2 for nn_DeTree (oblivious decision-tree routing).

Full-input contract: kernel(**inputs) takes unsharded inputs, returns the
full [2048, 512] output.  Internally batch-shards across NCORES NeuronCores
(every core runs all 512 trees on its batch slice).

All tree parameters are folded on host and baked into the NEFF as Const
(inline) DRAM tensors -- they are DMA'd to HBM once at model load, so per
execution the only shipped tensors are x (int8, host-pre-transposed, with
per-feature absmax scales as a tiny second input) -- dequantized to bf16
on the DVE before mm1.

Math per core (BC batch cols, 512 trees, nd rows = (t,d) pairs):
  W = softmax(fa, axis=0) * 0.5 * exp(-log_temp)   (host, bf16)
  u = W^T x + c0          (mm1, PE bf16; c0 = 0.5 - 0.5*thr*exp(-log_temp))
  up = clip(u, 0, 1)      (DVE)
  Lp = ln(up + d), Ln = ln((1+d) - up)             (ACT, bf16 out)
  S_lo/S_hi = path sums of logs (mm2, PE bf16 vs 0/1 matrices)
  e_lo = exp(S_lo) [8/tree], e_hi = exp(S_hi) [4/tree]
  z = R2 . e_lo           (mm3; response folded into R2)
  out = sum_hi e_hi * z   (DVE mul + mm4 ones-reduce)
Leaves split (lo: depths 0-2 -> 8 ids) x (hi: depths 3-4 -> 4 ids): 12 exps
per tree instead of 32.  delta=1e-6 stands in for the exact-0 gate (factor
1e-6 instead of 0 in the product -- negligible vs 2e-2 tolerance).
"""

import hashlib
import os
import sys

import numpy as np

for _p in ("/opt/trn_rl_repo", "/root/.axon_site/_ro/trn_rl_repo"):
    if os.path.isdir(_p) and _p not in sys.path:
        sys.path.append(_p)

import ml_dtypes

import concourse.bass as bass
import concourse.mybir as mybir
import concourse.tile as tile
from concourse.bass_utils import run_bass_kernel_spmd

F32 = mybir.dt.float32
BF16 = mybir.dt.bfloat16
AF = mybir.ActivationFunctionType
ALU = mybir.AluOpType
NPBF = ml_dtypes.bfloat16

# problem shape (hardcoded per contest contract)
B, F, N, D = 2048, 512, 512, 5
NLEAF = 32
NLO, NHI = 8, 4            # leaf-id split sizes (lo: depths 0-2, hi: 3-4)
NROW_LO = 3 * N            # 1536 lo rows (t-major, d=0..2 inner)
NROW_HI = 2 * N            # 1024 hi rows (t-major, d=3..4 inner)
ND = NROW_LO + NROW_HI     # 2560 = 20 tiles of 128
NT_LO = NROW_LO // 128     # 12
NT_ALL = ND // 128         # 20
NSG = N // 16              # 32 sigma groups (16 trees, 8 lo-ids)
NAL = N // 32              # 16 alpha groups (32 trees, 4 hi-ids)
NOM = N // 128             # 4 output tree-tiles
DELTA = 1e-6

NCORES = 8                 # batch-parallel cores
BJ = 256                   # batch chunk (free-axis) per pipeline pass

_CACHE = {}
LAST = None                # BassKernelResults of most recent run
FIXUP_WAITS = True         # False for CoreSim/TimelineSim


def _structure(path_map):
    """Path matrices + leaf regroup derived from the runtime path_map."""
    path = np.asarray(path_map).reshape(NLEAF, D)
    lo_t = [tuple(int(path[l, j]) for j in (0, 1, 2)) for l in range(NLEAF)]
    hi_t = [tuple(int(path[l, j]) for j in (3, 4)) for l in range(NLEAF)]
    lo_ids = sorted(set(lo_t))
    hi_ids = sorted(set(hi_t))
    assert len(lo_ids) <= NLO and len(hi_ids) <= NHI, "path_map does not factor"
    lo_of = {t: i for i, t in enumerate(lo_ids)}
    hi_of = {t: i for i, t in enumerate(hi_ids)}
    P_lo = np.zeros((2 * D, NLO), np.float32)
    for t, i in lo_of.items():
        for e in t:
            P_lo[e, i] += 1.0
    P_hi = np.zeros((2 * D, NHI), np.float32)
    for t, i in hi_of.items():
        for e in t:
            P_hi[e, i] += 1.0
    leaf_hi = np.array([hi_of[t] for t in hi_t], np.int64)
    leaf_lo = np.array([lo_of[t] for t in lo_t], np.int64)
    return P_lo, P_hi, leaf_hi, leaf_lo


def _mm2_passes(P_lo, P_hi):
    """lhsT tiles for the path-sum matmuls over all 512 trees.

    Returns (pb, sb_passes, sa_passes): pb [npb,128,128] f32;
    sb_passes[sigma] / sa_passes[alpha] lists of (pb_idx, tau, sign);
    tau indexes the 128-row nd-tile of Lp/Ln (lo tiles 0..11, hi 12..19).
    Out rows: S_lo sigma-tile partition 8*tl+lo (trees 16*sigma+tl);
    S_hi alpha-tile partition 4*tl+hi (trees 32*alpha+tl).
    """
    mats, sb_passes, sa_passes = [], [], []
    for sig in range(NSG):
        passes = []
        for s in (1, 0):
            by_tau = {}
            for tl in range(16):
                t = 16 * sig + tl
                for d in (0, 1, 2):
                    r = 3 * t + d
                    tau, k = r // 128, r % 128
                    m = by_tau.setdefault(tau, np.zeros((128, 128), np.float32))
                    for lo in range(NLO):
                        m[k, 8 * tl + lo] = P_lo[2 * d + s, lo]
            for tau in sorted(by_tau):
                passes.append((len(mats), tau, s))
                mats.append(by_tau[tau])
        sb_passes.append(passes)
    for al in range(NAL):
        passes = []
        for s in (1, 0):
            by_tau = {}
            for tl in range(32):
                t = 32 * al + tl
                for d in (3, 4):
                    r = 2 * t + (d - 3)
                    tau, k = NT_LO + r // 128, r % 128
                    m = by_tau.setdefault(tau, np.zeros((128, 128), np.float32))
                    for hi in range(NHI):
                        m[k, 4 * tl + hi] = P_hi[2 * d + s, hi]
            for tau in sorted(by_tau):
                passes.append((len(mats), tau, s))
                mats.append(by_tau[tau])
        sa_passes.append(passes)
    return np.stack(mats), sb_passes, sa_passes


def _host_consts(fa, thr, lt, resp, path_map):
    """All NEFF-baked constants, packed for single-DMA SBUF loads."""
    P_lo, P_hi, leaf_hi, leaf_lo = _structure(path_map)
    pb, sb_passes, sa_passes = _mm2_passes(P_lo, P_hi)

    # nd row order: lo-block rows 3t+d (d<3), hi-block rows 2t+(d-3)
    col = np.empty(ND, np.int64)
    for t in range(N):
        for d in range(3):
            col[3 * t + d] = 5 * t + d
        for d in (3, 4):
            col[NROW_LO + 2 * t + (d - 3)] = 5 * t + d

    E = np.exp(np.asarray(fa, np.float64))
    Wn = (E / E.sum(0)).astype(np.float64)                     # [F, N*D]
    invtemp = np.exp(-np.asarray(lt, np.float64)).reshape(N * D)
    Ws = (Wn * (0.5 * invtemp)).astype(np.float32)[:, col]     # [F, ND] permuted
    c0 = (0.5 - 0.5 * np.asarray(thr, np.float64).reshape(N * D) * invtemp
          ).astype(np.float32)[col]

    # Wpack [128, (nt*4+ft)*128 + j] = Ws[ft*128 + p, nt*128 + j]
    wpack = np.ascontiguousarray(
        Ws.reshape(4, 128, NT_ALL, 128).transpose(1, 2, 0, 3).reshape(128, -1)
    ).astype(NPBF)
    c0pack = np.ascontiguousarray(c0.reshape(NT_ALL, 128).T)   # [128, 20] f32

    pbpack = np.ascontiguousarray(
        pb.transpose(1, 0, 2).reshape(128, -1)).astype(NPBF)

    # R2[t, hi, lo] = sum of response over leaves in each (hi, lo) group
    R2 = np.zeros((N, NHI, NLO), np.float32)
    np.add.at(R2, (slice(None), leaf_hi, leaf_lo),
              np.asarray(resp, np.float32).reshape(N, NLEAF))
    # r2l sigma-block [8tl+lo, 64*(sigma%2) + 4tl+hi] = R2[16 sig + tl]
    r2l = np.zeros((NSG, 128, 128), np.float32)
    for sg in range(NSG):
        off = 64 * (sg % 2)
        for tl in range(16):
            for hi in range(NHI):
                for lo in range(NLO):
                    r2l[sg, 8 * tl + lo, off + 4 * tl + hi] = R2[16 * sg + tl, hi, lo]
    r2pack = np.ascontiguousarray(
        r2l.transpose(1, 0, 2).reshape(128, -1)).astype(NPBF)

    # onesd block j: [4tl+hi, 32j+tl] = 1   (alpha-to-tree reduce)
    onesd = np.zeros((4, 128, 128), np.float32)
    for j in range(4):
        for tl in range(32):
            for hi in range(NHI):
                onesd[j, 4 * tl + hi, 32 * j + tl] = 1.0
    onpack = np.ascontiguousarray(
        onesd.transpose(1, 0, 2).reshape(128, -1)).astype(NPBF)

    return wpack, c0pack, pbpack, r2pack, onpack, sb_passes, sa_passes


def _build_nc(consts):
    wpack, c0pack, pbpack, r2pack, onpack, sb_passes, sa_passes = consts
    BC = B // NCORES
    nchunk = BC // BJ
    npb = pbpack.shape[1] // 128

    nc = bass.Bass()
    # x ships int8 (halves upload bytes); per-feature dequant scales ride
    # along as a tiny second input (x-dependent, so not bakeable).
    x_in = nc.dram_tensor("x", [F, BC], mybir.dt.int8, kind="ExternalInput")
    xs_in = nc.dram_tensor("xs", [128, 4], F32, kind="ExternalInput")
    out_d = nc.dram_tensor("out_t", [N, BC], BF16, kind="ExternalOutput")
    wp_d = nc.inline_tensor(wpack, name="wp")
    c0_d = nc.inline_tensor(c0pack, name="c0p")
    pb_d = nc.inline_tensor(pbpack, name="pbp")
    r2_d = nc.inline_tensor(r2pack, name="r2p")
    on_d = nc.inline_tensor(onpack, name="onp")

    with tile.TileContext(nc) as tc:
        with (
            tc.tile_pool(name="const", bufs=1) as cpool,
            tc.tile_pool(name="chunk", bufs=1) as kpool,
            tc.tile_pool(name="work", bufs=3) as wpool,
            tc.tile_pool(name="out", bufs=2) as opool,
            tc.tile_pool(name="psA", bufs=2, space="PSUM") as ppa,
            tc.tile_pool(name="psZ", bufs=2, space="PSUM") as ppz,
            tc.tile_pool(name="psO", bufs=2, space="PSUM") as ppo,
        ):
            # ---- resident constants (one DMA each; W split so mm1 starts early)
            wpt = cpool.tile([128, NT_ALL * 4 * 128], BF16, tag="wp")
            half = NT_ALL * 4 * 128 // 2
            nc.sync.dma_start(wpt[:, 0:half], wp_d[:, 0:half])
            xst = cpool.tile([128, 4], F32, tag="xst")
            nc.sync.dma_start(xst[:], xs_in[:])
            xts = cpool.tile([128, 4 * BC], BF16, tag="xts")
            for ft in range(4):
                xqt = wpool.tile([128, BC], mybir.dt.int8, tag="xq")
                nc.sync.dma_start(xqt[:], x_in[bass.ts(ft, 128), :])
                nc.vector.tensor_scalar(xts[:, BC * ft:BC * (ft + 1)], xqt[:],
                                        xst[:, ft:ft + 1], None, ALU.mult)
            nc.sync.dma_start(wpt[:, half:], wp_d[:, half:])
            c0t = cpool.tile([128, NT_ALL], F32, tag="c0")
            nc.sync.dma_start(c0t[:], c0_d[:])
            bze = cpool.tile([128, 2], F32, tag="bze")
            nc.vector.memset(bze[:, 0:1], DELTA)
            nc.vector.memset(bze[:, 1:2], 1.0 + DELTA)
            pbt = cpool.tile([128, npb * 128], BF16, tag="pb")
            nc.sync.dma_start(pbt[:], pb_d[:])
            r2t = cpool.tile([128, NSG * 128], BF16, tag="r2")
            nc.sync.dma_start(r2t[:], r2_d[:])
            ont = cpool.tile([128, 4 * 128], BF16, tag="on")
            nc.sync.dma_start(ont[:], on_d[:])

            for j in range(nchunk):
                def xsl(ft):
                    return xts[:, BC * ft + BJ * j:BC * ft + BJ * j + BJ]

                lpt = kpool.tile([128, NT_ALL * BJ], BF16, tag="lp")
                lnt = kpool.tile([128, NT_ALL * BJ], BF16, tag="ln")
                ebt = kpool.tile([128, NSG * BJ], BF16, tag="eb")
                eat = kpool.tile([128, NAL * BJ], F32, tag="ea")

                # ---- mm1 + gates + logs ----
                for nt in range(NT_ALL):
                    fv = ppa.tile([128, BJ], F32, tag="ps")
                    for ft in range(4):
                        nc.tensor.matmul(
                            fv[:],
                            wpt[:, bass.ts(nt * 4 + ft, 128)],
                            xsl(ft),
                            start=ft == 0, stop=ft == 3)
                    u1 = wpool.tile([128, BJ], F32, tag="u1")
                    nc.vector.tensor_scalar(u1[:], fv[:], c0t[:, nt:nt + 1],
                                            1.0, ALU.add, ALU.min)
                    up0 = wpool.tile([128, BJ], F32, tag="u2")
                    nc.vector.tensor_scalar(up0[:], u1[:], 0.0, None, ALU.max)
                    nc.scalar.activation(lpt[:, bass.ts(nt, BJ)], up0[:],
                                         AF.Ln, bias=bze[:, 0:1], scale=1.0)
                    nc.scalar.activation(lnt[:, bass.ts(nt, BJ)], up0[:],
                                         AF.Ln, bias=bze[:, 1:2], scale=-1.0)

                def lsrc(s, tau):
                    src = lpt if s == 1 else lnt
                    return src[:, bass.ts(tau, BJ)]

                # ---- mm2 (path sums) + exp ----
                for sg in range(NSG):
                    sb = ppa.tile([128, BJ], F32, tag="ps")
                    passes = sb_passes[sg]
                    for i, (pi, tau, s) in enumerate(passes):
                        nc.tensor.matmul(
                            sb[:], pbt[:, bass.ts(pi, 128)], lsrc(s, tau),
                            start=i == 0, stop=i == len(passes) - 1)
                    nc.scalar.activation(ebt[:, bass.ts(sg, BJ)], sb[:], AF.Exp)
                for al in range(NAL):
                    sa = ppa.tile([128, BJ], F32, tag="ps")
                    passes = sa_passes[al]
                    for i, (pi, tau, s) in enumerate(passes):
                        nc.tensor.matmul(
                            sa[:], pbt[:, bass.ts(pi, 128)], lsrc(s, tau),
                            start=i == 0, stop=i == len(passes) - 1)
                    nc.scalar.activation(eat[:, bass.ts(al, BJ)], sa[:], AF.Exp)

                # ---- mm3 (z = R2 . e_lo), P = e_hi * z, mm4 (ones-reduce) ----
                for om in range(NOM):
                    ptb = kpool.tile([128, 4 * BJ], BF16, tag="ptb")
                    for jj in range(4):
                        al = 4 * om + jj
                        z = ppz.tile([128, BJ], F32, tag="z")
                        for k in range(2):
                            sg = 2 * al + k
                            nc.tensor.matmul(
                                z[:], r2t[:, bass.ts(sg, 128)],
                                ebt[:, bass.ts(sg, BJ)],
                                start=k == 0, stop=k == 1)
                        nc.vector.tensor_tensor(
                            ptb[:, bass.ts(jj, BJ)],
                            eat[:, bass.ts(al, BJ)], z[:], ALU.mult)
                    outp = ppo.tile([128, BJ], F32, tag="o")
                    for jj in range(4):
                        nc.tensor.matmul(
                            outp[:], ont[:, bass.ts(jj, 128)],
                            ptb[:, bass.ts(jj, BJ)],
                            start=jj == 0, stop=jj == 3)
                    osb = opool.tile([128, BJ], BF16, tag="osb")
                    nc.vector.tensor_copy(osb[:], outp[:])
                    nc.sync.dma_start(
                        out_d[bass.ts(om, 128), BJ * j:BJ * (j + 1)], osb[:])

    if FIXUP_WAITS:
        _split_excess_waits(nc)
    return nc


def _split_excess_waits(nc):
    """Walrus codegen only fits ONE sync wait on PE Matmult and DMACopy
    instructions ("Too many sync wait commands").  Hoist the extras onto
    preceding same-engine InstEventSemaphore pseudos (one wait each), which
    the sequencer executes before the limited instruction."""
    exempt = {"InstEventSemaphore", "InstUnconditionalBranch",
              "InstISA", "InstHalt"}
    nfix = 0
    for fn in nc.m.functions:
        for bb in fn.blocks:
            il = bb.instructions
            out = []
            for inst in il:
                si = inst.sync_info
                lim = None if type(inst).__name__ in exempt else 1
                if si is not None and lim is not None and len(si.on_wait) > lim:
                    keep = list(si.on_wait[-lim:])
                    for w in si.on_wait[:-lim]:
                        nfix += 1
                        ev = mybir.InstEventSemaphore(
                            name=f"I-waitfix-{nfix}",
                            engine=inst.engine,
                            ins=[], outs=[],
                            sync_info=mybir.SyncInfo(on_wait=[w], on_update=[]),
                        )
                        ev.bass_nofuse = True
                        out.append(ev)
                    inst.sync_info = mybir.SyncInfo(
                        on_wait=keep, on_update=list(si.on_update))
                out.append(inst)
            il[:] = out
            assert len(bb.instructions) == len(out)
    return nfix


def _prep(fa, thr, lt, resp, path_map):
    h = hashlib.sha256()
    for a in (fa, thr, lt, resp, path_map):
        h.update(np.ascontiguousarray(a).tobytes())
    key = (h.hexdigest(), NCORES, BJ)
    if key not in _CACHE:
        consts = _host_consts(fa, thr, lt, resp, path_map)
        nc = _build_nc(consts)
        _CACHE[key] = nc
    return _CACHE[key]


def build_in_maps(x, feat_attention, feature_thresholds, log_temperatures,
                  response, path_map):
    nc = _prep(np.asarray(feat_attention, np.float32),
               np.asarray(feature_thresholds, np.float32),
               np.asarray(log_temperatures, np.float32),
               np.asarray(response, np.float32),
               np.asarray(path_map))
    BC = B // NCORES
    xf = np.asarray(x, np.float32)
    scale = np.abs(xf).max(axis=0) / 127.0                  # per feature [F]
    scale[scale == 0.0] = 1.0
    xq = np.clip(np.rint(xf / scale), -127, 127).astype(np.int8)
    xT = np.ascontiguousarray(xq.T)                          # [F, B] int8
    xs = np.ascontiguousarray(scale.reshape(4, 128).T.astype(np.float32))
    in_maps = [{"x": np.ascontiguousarray(xT[:, c * BC:(c + 1) * BC]),
                "xs": xs}
               for c in range(NCORES)]
    return in_maps, nc


def kernel(x, feat_attention, feature_thresholds, log_temperatures,
           response, path_map):
    in_maps, nc = build_in_maps(x, feat_attention, feature_thresholds,
                                log_temperatures, response, path_map)
    res = run_bass_kernel_spmd(nc, in_maps, list(range(NCORES)))
    global LAST
    LAST = res
    BC = B // NCORES
    out = np.empty((B, N), np.float32)
    for c in range(NCORES):
        out[c * BC:(c + 1) * BC, :] = \
            np.asarray(res.results[c]["out_t"], np.float32).T
    return out



# revision 8
# speedup vs baseline: 5.9866x; 5.9866x over previous
"""Trainium2 Bass kernel v2 for nn_DeTree (oblivious decision-tree routing).

Full-input contract: kernel(**inputs) takes unsharded inputs, returns the
full [2048, 512] output.  Internally batch-shards across NCORES NeuronCores
(every core runs all 512 trees on its batch slice).

All tree parameters are folded on host and baked into the NEFF as Const
(inline) DRAM tensors -- they are DMA'd to HBM once at model load, so per
execution the only shipped tensors are x (int8, host-pre-transposed, with
per-feature absmax scales as a tiny second input) -- dequantized to bf16
on the DVE before mm1.

Math per core (BC batch cols, 512 trees, nd rows = (t,d) pairs):
  W = softmax(fa, axis=0) * 0.5 * exp(-log_temp)   (host, bf16)
  u = W^T x + c0          (mm1, PE bf16; c0 = 0.5 - 0.5*thr*exp(-log_temp))
  up = clip(u, 0, 1)      (DVE)
  Lp = ln(up + d), Ln = ln((1+d) - up)             (ACT, bf16 out)
  S_lo/S_hi = path sums of logs (mm2, PE bf16 vs 0/1 matrices)
  e_lo = exp(S_lo) [8/tree], e_hi = exp(S_hi) [4/tree]
  z = R2 . e_lo           (mm3; response folded into R2)
  out = sum_hi e_hi * z   (DVE mul + mm4 ones-reduce)
Leaves split (lo: depths 0-2 -> 8 ids) x (hi: depths 3-4 -> 4 ids): 12 exps
per tree instead of 32.  delta=1e-6 stands in for the exact-0 gate (factor
1e-6 instead of 0 in the product -- negligible vs 2e-2 tolerance).
"""

import hashlib
import os
import sys

import numpy as np

for _p in ("/opt/trn_rl_repo", "/root/.axon_site/_ro/trn_rl_repo"):
    if os.path.isdir(_p) and _p not in sys.path:
        sys.path.append(_p)

import ml_dtypes

import concourse.bass as bass
import concourse.mybir as mybir
import concourse.tile as tile
from concourse.bass_utils import run_bass_kernel_spmd

F32 = mybir.dt.float32
BF16 = mybir.dt.bfloat16
AF = mybir.ActivationFunctionType
ALU = mybir.AluOpType
NPBF = ml_dtypes.bfloat16

# problem shape (hardcoded per contest contract)
B, F, N, D = 2048, 512, 512, 5
NLEAF = 32
NLO, NHI = 8, 4            # leaf-id split sizes (lo: depths 0-2, hi: 3-4)
NROW_LO = 3 * N            # 1536 lo rows (t-major, d=0..2 inner)
NROW_HI = 2 * N            # 1024 hi rows (t-major, d=3..4 inner)
ND = NROW_LO + NROW_HI     # 2560 = 20 tiles of 128
NT_LO = NROW_LO // 128     # 12
NT_ALL = ND // 128         # 20
NSG = N // 16              # 32 sigma groups (16 trees, 8 lo-ids)
NAL = N // 32              # 16 alpha groups (32 trees, 4 hi-ids)
NOM = N // 128             # 4 output tree-tiles
DELTA = 1e-6

NCORES = 8                 # batch-parallel cores
BJ = 256                   # batch chunk (free-axis) per pipeline pass

_CACHE = {}
LAST = None                # BassKernelResults of most recent run
FIXUP_WAITS = True         # False for CoreSim/TimelineSim


def _structure(path_map):
    """Path matrices + leaf regroup derived from the runtime path_map."""
    path = np.asarray(path_map).reshape(NLEAF, D)
    lo_t = [tuple(int(path[l, j]) for j in (0, 1, 2)) for l in range(NLEAF)]
    hi_t = [tuple(int(path[l, j]) for j in (3, 4)) for l in range(NLEAF)]
    lo_ids = sorted(set(lo_t))
    hi_ids = sorted(set(hi_t))
    assert len(lo_ids) <= NLO and len(hi_ids) <= NHI, "path_map does not factor"
    lo_of = {t: i for i, t in enumerate(lo_ids)}
    hi_of = {t: i for i, t in enumerate(hi_ids)}
    P_lo = np.zeros((2 * D, NLO), np.float32)
    for t, i in lo_of.items():
        for e in t:
            P_lo[e, i] += 1.0
    P_hi = np.zeros((2 * D, NHI), np.float32)
    for t, i in hi_of.items():
        for e in t:
            P_hi[e, i] += 1.0
    leaf_hi = np.array([hi_of[t] for t in hi_t], np.int64)
    leaf_lo = np.array([lo_of[t] for t in lo_t], np.int64)
    return P_lo, P_hi, leaf_hi, leaf_lo


def _mm2_passes(P_lo, P_hi):
    """lhsT tiles for the path-sum matmuls over all 512 trees.

    Returns (pb, sb_passes, sa_passes): pb [npb,128,128] f32;
    sb_passes[sigma] / sa_passes[alpha] lists of (pb_idx, tau, sign);
    tau indexes the 128-row nd-tile of Lp/Ln (lo tiles 0..11, hi 12..19).
    Out rows: S_lo sigma-tile partition 8*tl+lo (trees 16*sigma+tl);
    S_hi alpha-tile partition 4*tl+hi (trees 32*alpha+tl).
    """
    mats, sb_passes, sa_passes = [], [], []
    for sig in range(NSG):
        passes = []
        for s in (1, 0):
            by_tau = {}
            for tl in range(16):
                t = 16 * sig + tl
                for d in (0, 1, 2):
                    r = 3 * t + d
                    tau, k = r // 128, r % 128
                    m = by_tau.setdefault(tau, np.zeros((128, 128), np.float32))
                    for lo in range(NLO):
                        m[k, 8 * tl + lo] = P_lo[2 * d + s, lo]
            for tau in sorted(by_tau):
                passes.append((len(mats), tau, s))
                mats.append(by_tau[tau])
        sb_passes.append(passes)
    for al in range(NAL):
        # n-basis alpha pass: S_n[4tl+j] = sum_{d in hi-set(j)} Lp(t,d)
        # j: 0 -> {}, 1 -> {3}, 2 -> {4}, 3 -> {3,4}; single Lp-only matmul
        by_tau = {}
        for tl in range(32):
            t = 32 * al + tl
            for j, ds in enumerate(((), (3,), (4,), (3, 4))):
                for d in ds:
                    r = 2 * t + (d - 3)
                    tau, k = NT_LO + r // 128, r % 128
                    m = by_tau.setdefault(tau, np.zeros((128, 128), np.float32))
                    m[k, 4 * tl + j] = 1.0
        passes = [(len(mats), tau, 1) for tau in sorted(by_tau)]
        for tau in sorted(by_tau):
            mats.append(by_tau[tau])
        sa_passes.append(passes)
    return np.stack(mats), sb_passes, sa_passes


def _host_consts(fa, thr, lt, resp, path_map):
    """All NEFF-baked constants, packed for single-DMA SBUF loads."""
    P_lo, P_hi, leaf_hi, leaf_lo = _structure(path_map)
    pb, sb_passes, sa_passes = _mm2_passes(P_lo, P_hi)

    # nd row order: lo-block rows 3t+d (d<3), hi-block rows 2t+(d-3)
    col = np.empty(ND, np.int64)
    for t in range(N):
        for d in range(3):
            col[3 * t + d] = 5 * t + d
        for d in (3, 4):
            col[NROW_LO + 2 * t + (d - 3)] = 5 * t + d

    E = np.exp(np.asarray(fa, np.float64))
    Wn = (E / E.sum(0)).astype(np.float64)                     # [F, N*D]
    invtemp = np.exp(-np.asarray(lt, np.float64)).reshape(N * D)
    Ws = (Wn * (0.5 * invtemp)).astype(np.float32)[:, col]     # [F, ND] permuted
    c0 = (0.5 - 0.5 * np.asarray(thr, np.float64).reshape(N * D) * invtemp
          ).astype(np.float32)[col]

    # Wpack [128, (nt*4+ft)*128 + j] = Ws[ft*128 + p, nt*128 + j]
    wpack = np.ascontiguousarray(
        Ws.reshape(4, 128, NT_ALL, 128).transpose(1, 2, 0, 3).reshape(128, -1)
    ).astype(NPBF)
    c0pack = np.ascontiguousarray(c0.reshape(NT_ALL, 128).T)   # [128, 20] f32

    pbpack = np.ascontiguousarray(
        pb.transpose(1, 0, 2).reshape(128, -1)).astype(NPBF)

    # R2[t, hi, lo] = sum of response over leaves in each (hi, lo) group
    R2 = np.zeros((N, NHI, NLO), np.float64)
    np.add.at(R2, (slice(None), leaf_hi, leaf_lo),
              np.asarray(resp, np.float64).reshape(N, NLEAF))
    # hi side in monomial basis: e_hi = sum_j M[hi,j] * n_j with
    # n = {1, u3, u4, u3*u4}; fold M into the mm3 matrix (K' replaces R2)
    path = np.asarray(path_map).reshape(NLEAF, D)
    hi_ids = sorted(set(tuple(int(path[l, j]) for j in (3, 4))
                        for l in range(NLEAF)))
    M = np.zeros((NHI, NHI))
    for i, (e3, e4) in enumerate(hi_ids):
        b3, b4 = e3 % 2, e4 % 2
        a3 = (0.0, 1.0) if b3 else (1.0, -1.0)   # (const, u3) coeffs
        a4 = (0.0, 1.0) if b4 else (1.0, -1.0)
        M[i, 0] = a3[0] * a4[0]
        M[i, 1] = a3[1] * a4[0]
        M[i, 2] = a3[0] * a4[1]
        M[i, 3] = a3[1] * a4[1]
    R2 = np.einsum('hj,nhl->njl', M, R2).astype(np.float32)
    # r2l sigma-block [8tl+lo, 64*(sigma%2) + 4tl+hi] = R2[16 sig + tl]
    r2l = np.zeros((NSG, 128, 128), np.float32)
    for sg in range(NSG):
        off = 64 * (sg % 2)
        for tl in range(16):
            for hi in range(NHI):
                for lo in range(NLO):
                    r2l[sg, 8 * tl + lo, off + 4 * tl + hi] = R2[16 * sg + tl, hi, lo]
    r2pack = np.ascontiguousarray(
        r2l.transpose(1, 0, 2).reshape(128, -1)).astype(NPBF)

    # onesd block j: [4tl+hi, 32j+tl] = 1   (alpha-to-tree reduce)
    onesd = np.zeros((4, 128, 128), np.float32)
    for j in range(4):
        for tl in range(32):
            for hi in range(NHI):
                onesd[j, 4 * tl + hi, 32 * j + tl] = 1.0
    onpack = np.ascontiguousarray(
        onesd.transpose(1, 0, 2).reshape(128, -1)).astype(NPBF)

    return wpack, c0pack, pbpack, r2pack, onpack, sb_passes, sa_passes


def _build_nc(consts):
    wpack, c0pack, pbpack, r2pack, onpack, sb_passes, sa_passes = consts
    BC = B // NCORES
    nchunk = BC // BJ
    npb = pbpack.shape[1] // 128

    nc = bass.Bass()
    # x ships int8 (halves upload bytes); per-feature dequant scales ride
    # along as a tiny second input (x-dependent, so not bakeable).
    x_in = nc.dram_tensor("x", [F, BC], mybir.dt.int8, kind="ExternalInput")
    xs_in = nc.dram_tensor("xs", [128, 4], F32, kind="ExternalInput")
    out_d = nc.dram_tensor("out_t", [N, BC], BF16, kind="ExternalOutput")
    wp_d = nc.inline_tensor(wpack, name="wp")
    c0_d = nc.inline_tensor(c0pack, name="c0p")
    pb_d = nc.inline_tensor(pbpack, name="pbp")
    r2_d = nc.inline_tensor(r2pack, name="r2p")
    on_d = nc.inline_tensor(onpack, name="onp")

    with tile.TileContext(nc) as tc:
        with (
            tc.tile_pool(name="const", bufs=1) as cpool,
            tc.tile_pool(name="chunk", bufs=1) as kpool,
            tc.tile_pool(name="work", bufs=3) as wpool,
            tc.tile_pool(name="upg", bufs=2) as upool,
            tc.tile_pool(name="out", bufs=2) as opool,
            tc.tile_pool(name="psA", bufs=2, space="PSUM") as ppa,
            tc.tile_pool(name="psS", bufs=2, space="PSUM") as pps,
            tc.tile_pool(name="psZ", bufs=1, space="PSUM") as ppz,
            tc.tile_pool(name="psO", bufs=1, space="PSUM") as ppo,
        ):
            # ---- resident constants (one DMA each; W split so mm1 starts early)
            wpt = cpool.tile([128, NT_ALL * 4 * 128], BF16, tag="wp")
            half = NT_ALL * 4 * 128 // 2
            nc.sync.dma_start(wpt[:, 0:half], wp_d[:, 0:half])
            xst = cpool.tile([128, 4], F32, tag="xst")
            nc.sync.dma_start(xst[:], xs_in[:])
            xts = cpool.tile([128, 4 * BC], BF16, tag="xts")
            for ft in range(4):
                xqt = wpool.tile([128, BC], mybir.dt.int8, tag="xq")
                nc.sync.dma_start(xqt[:], x_in[bass.ts(ft, 128), :])
                nc.vector.tensor_scalar(xts[:, BC * ft:BC * (ft + 1)], xqt[:],
                                        xst[:, ft:ft + 1], None, ALU.mult)
            nc.sync.dma_start(wpt[:, half:], wp_d[:, half:])
            c0t = cpool.tile([128, NT_ALL], F32, tag="c0")
            nc.sync.dma_start(c0t[:], c0_d[:])
            bze = cpool.tile([128, 2], F32, tag="bze")
            nc.vector.memset(bze[:, 0:1], DELTA)
            nc.vector.memset(bze[:, 1:2], 1.0 + DELTA)
            pbt = cpool.tile([128, npb * 128], BF16, tag="pb")
            nc.sync.dma_start(pbt[:], pb_d[:])
            r2t = cpool.tile([128, NSG * 128], BF16, tag="r2")
            nc.sync.dma_start(r2t[:], r2_d[:])
            ont = cpool.tile([128, 4 * 128], BF16, tag="on")
            nc.sync.dma_start(ont[:], on_d[:])

            for j in [jj % nchunk for jj in range(REPEAT * nchunk)]:
                def xsl(ft):
                    return xts[:, BC * ft + BJ * j:BC * ft + BJ * j + BJ]

                lpt = kpool.tile([128, NT_ALL * BJ], BF16, tag="lp")
                lnt = kpool.tile([128, NT_LO * BJ], BF16, tag="ln")
                ebt = kpool.tile([128, NSG * BJ], BF16, tag="eb")
                eat = kpool.tile([128, NAL * BJ], F32, tag="ea")

                # ---- mm1 + gates + logs (ACT fused over GF nd-tiles) ----
                GF = 5
                for g in range(NT_ALL // GF):
                    upg = upool.tile([128, GF * BJ], BF16, tag="upg")
                    for k in range(GF):
                        nt = g * GF + k
                        fv = ppa.tile([128, BJ], F32, tag="ps")
                        for ft in range(4):
                            nc.tensor.matmul(
                                fv[:],
                                wpt[:, bass.ts(nt * 4 + ft, 128)],
                                xsl(ft),
                                start=ft == 0, stop=ft == 3)
                        u1 = wpool.tile([128, BJ], F32, tag="u1")
                        nc.vector.tensor_scalar(u1[:], fv[:],
                                                c0t[:, nt:nt + 1],
                                                1.0, ALU.add, ALU.min)
                        nc.vector.tensor_scalar(upg[:, bass.ts(k, BJ)],
                                                u1[:], 0.0, None, ALU.max)
                    nc.scalar.activation(
                        lpt[:, g * GF * BJ:(g + 1) * GF * BJ], upg[:],
                        AF.Ln, bias=bze[:, 0:1], scale=1.0)
                    nlo = min(max(NT_LO - g * GF, 0), GF)   # lo tiles in group
                    if nlo:
                        nc.scalar.activation(
                            lnt[:, g * GF * BJ:(g * GF + nlo) * BJ],
                            upg[:, 0:nlo * BJ],
                            AF.Ln, bias=bze[:, 1:2], scale=-1.0)

                def lsrc(s, tau):
                    src = lpt if s == 1 else lnt
                    return src[:, bass.ts(tau, BJ)]

                # ---- mm2 (path sums) + exp (fused over pairs of groups) ----
                for sg2 in range(NSG // 2):
                    sb2 = pps.tile([128, 2 * BJ], F32, tag="S")
                    for k in range(2):
                        passes = sb_passes[2 * sg2 + k]
                        for i, (pi, tau, s) in enumerate(passes):
                            nc.tensor.matmul(
                                sb2[:, bass.ts(k, BJ)],
                                pbt[:, bass.ts(pi, 128)], lsrc(s, tau),
                                start=i == 0, stop=i == len(passes) - 1)
                    nc.scalar.activation(ebt[:, bass.ts(sg2, 2 * BJ)],
                                         sb2[:], AF.Exp)
                for al2 in range(NAL // 2):
                    sa2 = pps.tile([128, 2 * BJ], F32, tag="S")
                    for k in range(2):
                        passes = sa_passes[2 * al2 + k]
                        for i, (pi, tau, s) in enumerate(passes):
                            nc.tensor.matmul(
                                sa2[:, bass.ts(k, BJ)],
                                pbt[:, bass.ts(pi, 128)], lsrc(s, tau),
                                start=i == 0, stop=i == len(passes) - 1)
                    nc.scalar.activation(eat[:, bass.ts(al2, 2 * BJ)],
                                         sa2[:], AF.Exp)

                # ---- mm3 (z = R2 . e_lo), P = e_hi * z, mm4 (ones-reduce) ----
                for om in range(NOM):
                    ptb = kpool.tile([128, 4 * BJ], BF16, tag="ptb")
                    for jj in range(4):
                        al = 4 * om + jj
                        z = ppz.tile([128, BJ], F32, tag="z")
                        for k in range(2):
                            sg = 2 * al + k
                            nc.tensor.matmul(
                                z[:], r2t[:, bass.ts(sg, 128)],
                                ebt[:, bass.ts(sg, BJ)],
                                start=k == 0, stop=k == 1)
                        nc.vector.tensor_tensor(
                            ptb[:, bass.ts(jj, BJ)],
                            eat[:, bass.ts(al, BJ)], z[:], ALU.mult)
                    outp = ppo.tile([128, BJ], F32, tag="o")
                    for jj in range(4):
                        nc.tensor.matmul(
                            outp[:], ont[:, bass.ts(jj, 128)],
                            ptb[:, bass.ts(jj, BJ)],
                            start=jj == 0, stop=jj == 3)
                    osb = opool.tile([128, BJ], BF16, tag="osb")
                    nc.vector.tensor_copy(osb[:], outp[:])
                    nc.sync.dma_start(
                        out_d[bass.ts(om, 128), BJ * j:BJ * (j + 1)], osb[:])

    if FIXUP_WAITS:
        _split_excess_waits(nc)
    return nc


def _split_excess_waits(nc):
    """Walrus codegen only fits ONE sync wait on PE Matmult and DMACopy
    instructions ("Too many sync wait commands").  Hoist the extras onto
    preceding same-engine InstEventSemaphore pseudos (one wait each), which
    the sequencer executes before the limited instruction."""
    exempt = {"InstEventSemaphore", "InstUnconditionalBranch",
              "InstISA", "InstHalt"}
    nfix = 0
    for fn in nc.m.functions:
        for bb in fn.blocks:
            il = bb.instructions
            out = []
            for inst in il:
                si = inst.sync_info
                lim = None if type(inst).__name__ in exempt else 1
                if si is not None and lim is not None and len(si.on_wait) > lim:
                    keep = list(si.on_wait[-lim:])
                    for w in si.on_wait[:-lim]:
                        nfix += 1
                        ev = mybir.InstEventSemaphore(
                            name=f"I-waitfix-{nfix}",
                            engine=inst.engine,
                            ins=[], outs=[],
                            sync_info=mybir.SyncInfo(on_wait=[w], on_update=[]),
                        )
                        ev.bass_nofuse = True
                        out.append(ev)
                    inst.sync_info = mybir.SyncInfo(
                        on_wait=keep, on_update=list(si.on_update))
                out.append(inst)
            il[:] = out
            assert len(bb.instructions) == len(out)
    return nfix


def _prep(fa, thr, lt, resp, path_map):
    h = hashlib.sha256()
    for a in (fa, thr, lt, resp, path_map):
        h.update(np.ascontiguousarray(a).tobytes())
    key = (h.hexdigest(), NCORES, BJ, REPEAT)
    if key not in _CACHE:
        consts = _host_consts(fa, thr, lt, resp, path_map)
        nc = _build_nc(consts)
        _CACHE[key] = nc
    return _CACHE[key]


def build_in_maps(x, feat_attention, feature_thresholds, log_temperatures,
                  response, path_map):
    nc = _prep(np.asarray(feat_attention, np.float32),
               np.asarray(feature_thresholds, np.float32),
               np.asarray(log_temperatures, np.float32),
               np.asarray(response, np.float32),
               np.asarray(path_map))
    BC = B // NCORES
    xf = np.asarray(x, np.float32)
    scale = np.abs(xf).max(axis=0) / 127.0                  # per feature [F]
    scale[scale == 0.0] = 1.0
    xq = np.clip(np.rint(xf / scale), -127, 127).astype(np.int8)
    xT = np.ascontiguousarray(xq.T)                          # [F, B] int8
    xs = np.ascontiguousarray(scale.reshape(4, 128).T.astype(np.float32))
    in_maps = [{"x": np.ascontiguousarray(xT[:, c * BC:(c + 1) * BC]),
                "xs": xs}
               for c in range(NCORES)]
    return in_maps, nc


def kernel(x, feat_attention, feature_thresholds, log_temperatures,
           response, path_map):
    in_maps, nc = build_in_maps(x, feat_attention, feature_thresholds,
                                log_temperatures, response, path_map)
    res = run_bass_kernel_spmd(nc, in_maps, list(range(NCORES)))
    global LAST
    LAST = res
    BC = B // NCORES
    out = np.empty((B, N), np.float32)
    for c in range(NCORES):
        out[c * BC:(c + 1) * BC, :] = \
            np.asarray(res.results[c]["out_t"], np.float32).T
    return out



# revision 10
# speedup vs baseline: 6.7624x; 1.1296x over previous
"""Trainium2 Bass kernel v2 for nn_DeTree (oblivious decision-tree routing).

Full-input contract: kernel(**inputs) takes unsharded inputs, returns the
full [2048, 512] output.  Internally batch-shards across NCORES NeuronCores
(every core runs all 512 trees on its batch slice).

All tree parameters are folded on host and baked into the NEFF as Const
(inline) DRAM tensors -- they are DMA'd to HBM once at model load, so per
execution the only shipped tensors are x (int8, host-pre-transposed, with
per-feature absmax scales as a tiny second input) -- dequantized to bf16
on the DVE before mm1.

Math per core (BC batch cols, 512 trees, nd rows = (t,d) pairs):
  W = softmax(fa, axis=0) * 0.5 * exp(-log_temp)   (host, bf16)
  u = W^T x + c0          (mm1, PE bf16; c0 = 0.5 - 0.5*thr*exp(-log_temp))
  up = clip(u, 0, 1)      (DVE)
  Lp = ln(up + d), Ln = ln((1+d) - up)             (ACT, bf16 out)
  S_lo/S_hi = path sums of logs (mm2, PE bf16 vs 0/1 matrices)
  e_lo = exp(S_lo) [8/tree], e_hi = exp(S_hi) [4/tree]
  z = R2 . e_lo           (mm3; response folded into R2)
  out = sum_hi e_hi * z   (DVE mul + mm4 ones-reduce)
Leaves split (lo: depths 0-2 -> 8 ids) x (hi: depths 3-4 -> 4 ids): 12 exps
per tree instead of 32.  delta=1e-6 stands in for the exact-0 gate (factor
1e-6 instead of 0 in the product -- negligible vs 2e-2 tolerance).
"""

import hashlib
import os
import sys

import numpy as np

for _p in ("/opt/trn_rl_repo", "/root/.axon_site/_ro/trn_rl_repo"):
    if os.path.isdir(_p) and _p not in sys.path:
        sys.path.append(_p)

import ml_dtypes

import concourse.bass as bass
import concourse.mybir as mybir
import concourse.tile as tile
from concourse.bass_utils import run_bass_kernel_spmd

F32 = mybir.dt.float32
BF16 = mybir.dt.bfloat16
AF = mybir.ActivationFunctionType
ALU = mybir.AluOpType
NPBF = ml_dtypes.bfloat16

# problem shape (hardcoded per contest contract)
B, F, N, D = 2048, 512, 512, 5
NLEAF = 32
NLO, NHI = 8, 4            # leaf-id split sizes (lo: depths 0-2, hi: 3-4)
NROW_LO = 3 * N            # 1536 lo rows (t-major, d=0..2 inner)
NROW_HI = 2 * N            # 1024 hi rows (t-major, d=3..4 inner)
ND = NROW_LO + NROW_HI     # 2560 = 20 tiles of 128
NT_LO = NROW_LO // 128     # 12
NT_ALL = ND // 128         # 20
NSG = N // 16              # 32 sigma groups (16 trees, 8 lo-ids)
NAL = N // 32              # 16 alpha groups (32 trees, 4 hi-ids)
NOM = N // 128             # 4 output tree-tiles
DELTA = 1e-6

NCORES = 8                 # batch-parallel cores
BJ = 256                   # batch chunk (free-axis) per pipeline pass

_CACHE = {}
LAST = None                # BassKernelResults of most recent run
FIXUP_WAITS = True         # False for CoreSim/TimelineSim


def _structure(path_map):
    """Path matrices + leaf regroup derived from the runtime path_map."""
    path = np.asarray(path_map).reshape(NLEAF, D)
    lo_t = [tuple(int(path[l, j]) for j in (0, 1, 2)) for l in range(NLEAF)]
    hi_t = [tuple(int(path[l, j]) for j in (3, 4)) for l in range(NLEAF)]
    lo_ids = sorted(set(lo_t))
    hi_ids = sorted(set(hi_t))
    assert len(lo_ids) <= NLO and len(hi_ids) <= NHI, "path_map does not factor"
    lo_of = {t: i for i, t in enumerate(lo_ids)}
    hi_of = {t: i for i, t in enumerate(hi_ids)}
    P_lo = np.zeros((2 * D, NLO), np.float32)
    for t, i in lo_of.items():
        for e in t:
            P_lo[e, i] += 1.0
    P_hi = np.zeros((2 * D, NHI), np.float32)
    for t, i in hi_of.items():
        for e in t:
            P_hi[e, i] += 1.0
    leaf_hi = np.array([hi_of[t] for t in hi_t], np.int64)
    leaf_lo = np.array([lo_of[t] for t in lo_t], np.int64)
    return P_lo, P_hi, leaf_hi, leaf_lo


def _mm2_passes(P_lo, P_hi):
    """lhsT tiles for the path-sum matmuls over all 512 trees.

    Returns (pb, sb_passes, sa_passes): pb [npb,128,128] f32;
    sb_passes[sigma] / sa_passes[alpha] lists of (pb_idx, tau, sign);
    tau indexes the 128-row nd-tile of Lp/Ln (lo tiles 0..11, hi 12..19).
    Out rows: S_lo sigma-tile partition 8*tl+lo (trees 16*sigma+tl);
    S_hi alpha-tile partition 4*tl+hi (trees 32*alpha+tl).
    """
    mats, sb_passes, sa_passes = [], [], []
    for sig in range(NSG):
        passes = []
        for s in (1, 0):
            by_tau = {}
            for tl in range(16):
                t = 16 * sig + tl
                for d in (0, 1, 2):
                    r = 3 * t + d
                    tau, k = r // 128, r % 128
                    m = by_tau.setdefault(tau, np.zeros((128, 128), np.float32))
                    for lo in range(NLO):
                        m[k, 8 * tl + lo] = P_lo[2 * d + s, lo]
            for tau in sorted(by_tau):
                passes.append((len(mats), tau, s))
                mats.append(by_tau[tau])
        sb_passes.append(passes)
    for al in range(NAL):
        # n-basis alpha pass: S_n[4tl+j] = sum_{d in hi-set(j)} Lp(t,d)
        # j: 0 -> {}, 1 -> {3}, 2 -> {4}, 3 -> {3,4}; single Lp-only matmul
        by_tau = {}
        for tl in range(32):
            t = 32 * al + tl
            for j, ds in enumerate(((), (3,), (4,), (3, 4))):
                for d in ds:
                    r = 2 * t + (d - 3)
                    tau, k = NT_LO + r // 128, r % 128
                    m = by_tau.setdefault(tau, np.zeros((128, 128), np.float32))
                    m[k, 4 * tl + j] = 1.0
        passes = [(len(mats), tau, 1) for tau in sorted(by_tau)]
        for tau in sorted(by_tau):
            mats.append(by_tau[tau])
        sa_passes.append(passes)
    return np.stack(mats), sb_passes, sa_passes


def _host_consts(fa, thr, lt, resp, path_map):
    """All NEFF-baked constants, packed for single-DMA SBUF loads."""
    P_lo, P_hi, leaf_hi, leaf_lo = _structure(path_map)
    pb, sb_passes, sa_passes = _mm2_passes(P_lo, P_hi)

    # nd row order: lo-block rows 3t+d (d<3), hi-block rows 2t+(d-3)
    col = np.empty(ND, np.int64)
    for t in range(N):
        for d in range(3):
            col[3 * t + d] = 5 * t + d
        for d in (3, 4):
            col[NROW_LO + 2 * t + (d - 3)] = 5 * t + d

    E = np.exp(np.asarray(fa, np.float64))
    Wn = (E / E.sum(0)).astype(np.float64)                     # [F, N*D]
    invtemp = np.exp(-np.asarray(lt, np.float64)).reshape(N * D)
    Ws = (Wn * (0.5 * invtemp)).astype(np.float32)[:, col]     # [F, ND] permuted
    c0 = (0.5 - 0.5 * np.asarray(thr, np.float64).reshape(N * D) * invtemp
          ).astype(np.float32)[col]

    # Wpack [128, (nt*4+ft)*128 + j] = Ws[ft*128 + p, nt*128 + j]
    wpack = np.ascontiguousarray(
        Ws.reshape(4, 128, NT_ALL, 128).transpose(1, 2, 0, 3).reshape(128, -1)
    ).astype(NPBF)
    c0pack = np.ascontiguousarray(c0.reshape(NT_ALL, 128).T)   # [128, 20] f32

    pbpack = np.ascontiguousarray(
        pb.transpose(1, 0, 2).reshape(128, -1)).astype(NPBF)

    # R2[t, hi, lo] = sum of response over leaves in each (hi, lo) group
    R2 = np.zeros((N, NHI, NLO), np.float64)
    np.add.at(R2, (slice(None), leaf_hi, leaf_lo),
              np.asarray(resp, np.float64).reshape(N, NLEAF))
    # hi side in monomial basis: e_hi = sum_j M[hi,j] * n_j with
    # n = {1, u3, u4, u3*u4}; fold M into the mm3 matrix (K' replaces R2)
    path = np.asarray(path_map).reshape(NLEAF, D)
    hi_ids = sorted(set(tuple(int(path[l, j]) for j in (3, 4))
                        for l in range(NLEAF)))
    M = np.zeros((NHI, NHI))
    for i, (e3, e4) in enumerate(hi_ids):
        b3, b4 = e3 % 2, e4 % 2
        a3 = (0.0, 1.0) if b3 else (1.0, -1.0)   # (const, u3) coeffs
        a4 = (0.0, 1.0) if b4 else (1.0, -1.0)
        M[i, 0] = a3[0] * a4[0]
        M[i, 1] = a3[1] * a4[0]
        M[i, 2] = a3[0] * a4[1]
        M[i, 3] = a3[1] * a4[1]
    R2 = np.einsum('hj,nhl->njl', M, R2).astype(np.float32)
    # r2l sigma-block [8tl+lo, 64*(sigma%2) + 4tl+hi] = R2[16 sig + tl]
    r2l = np.zeros((NSG, 128, 128), np.float32)
    for sg in range(NSG):
        off = 64 * (sg % 2)
        for tl in range(16):
            for hi in range(NHI):
                for lo in range(NLO):
                    r2l[sg, 8 * tl + lo, off + 4 * tl + hi] = R2[16 * sg + tl, hi, lo]
    r2pack = np.ascontiguousarray(
        r2l.transpose(1, 0, 2).reshape(128, -1)).astype(NPBF)

    # onesd block j: [4tl+hi, 32j+tl] = 1   (alpha-to-tree reduce)
    onesd = np.zeros((4, 128, 128), np.float32)
    for j in range(4):
        for tl in range(32):
            for hi in range(NHI):
                onesd[j, 4 * tl + hi, 32 * j + tl] = 1.0
    onpack = np.ascontiguousarray(
        onesd.transpose(1, 0, 2).reshape(128, -1)).astype(NPBF)

    return wpack, c0pack, pbpack, r2pack, onpack, sb_passes, sa_passes


def _build_nc(consts):
    wpack, c0pack, pbpack, r2pack, onpack, sb_passes, sa_passes = consts
    BC = B // NCORES
    nchunk = BC // BJ
    npb = pbpack.shape[1] // 128

    nc = bass.Bass()
    # x ships int8 (halves upload bytes); per-feature dequant scales ride
    # along as a tiny second input (x-dependent, so not bakeable).
    x_in = nc.dram_tensor("x", [F, BC], mybir.dt.int8, kind="ExternalInput")
    xs_in = nc.dram_tensor("xs", [128, 4], F32, kind="ExternalInput")
    out_d = nc.dram_tensor("out_t", [N, BC], BF16, kind="ExternalOutput")
    wp_d = nc.inline_tensor(wpack, name="wp")
    c0_d = nc.inline_tensor(c0pack, name="c0p")
    pb_d = nc.inline_tensor(pbpack, name="pbp")
    r2_d = nc.inline_tensor(r2pack, name="r2p")
    on_d = nc.inline_tensor(onpack, name="onp")

    with tile.TileContext(nc) as tc:
        with (
            tc.tile_pool(name="const", bufs=1) as cpool,
            tc.tile_pool(name="chunk", bufs=2) as kpool,
            tc.tile_pool(name="epool", bufs=3) as epool,
            tc.tile_pool(name="work", bufs=5) as wpool,
            tc.tile_pool(name="upg", bufs=2) as upool,
            tc.tile_pool(name="out", bufs=2) as opool,
            tc.tile_pool(name="psA", bufs=2, space="PSUM") as ppa,
            tc.tile_pool(name="psS", bufs=2, space="PSUM") as pps,
            tc.tile_pool(name="psZ", bufs=1, space="PSUM") as ppz,
            tc.tile_pool(name="psO", bufs=1, space="PSUM") as ppo,
        ):
            # ---- resident constants (one DMA each; W split so mm1 starts early)
            wpt = cpool.tile([128, NT_ALL * 4 * 128], BF16, tag="wp")
            half = NT_ALL * 4 * 128 // 2
            nc.sync.dma_start(wpt[:, 0:half], wp_d[:, 0:half])
            xst = cpool.tile([128, 4], F32, tag="xst")
            nc.sync.dma_start(xst[:], xs_in[:])
            xts = cpool.tile([128, 4 * BC], BF16, tag="xts")
            for ft in range(4):
                xqt = wpool.tile([128, BC], mybir.dt.int8, tag="xq")
                nc.sync.dma_start(xqt[:], x_in[bass.ts(ft, 128), :])
                nc.vector.tensor_scalar(xts[:, BC * ft:BC * (ft + 1)], xqt[:],
                                        xst[:, ft:ft + 1], None, ALU.mult)
            nc.sync.dma_start(wpt[:, half:], wp_d[:, half:])
            c0t = cpool.tile([128, NT_ALL], F32, tag="c0")
            nc.sync.dma_start(c0t[:], c0_d[:])
            bze = cpool.tile([128, 2], F32, tag="bze")
            nc.vector.memset(bze[:, 0:1], DELTA)
            nc.vector.memset(bze[:, 1:2], 1.0 + DELTA)
            pbt = cpool.tile([128, npb * 128], BF16, tag="pb")
            nc.sync.dma_start(pbt[:], pb_d[:])
            r2t = cpool.tile([128, NSG * 128], BF16, tag="r2")
            nc.sync.dma_start(r2t[:], r2_d[:])
            ont = cpool.tile([128, 4 * 128], BF16, tag="on")
            nc.sync.dma_start(ont[:], on_d[:])

            for j in [jj % nchunk for jj in range(REPEAT * nchunk)]:
                def xsl(ft):
                    return xts[:, BC * ft + BJ * j:BC * ft + BJ * j + BJ]

                lpt = kpool.tile([128, NT_ALL * BJ], BF16, tag="lp")
                lnt = kpool.tile([128, NT_LO * BJ], BF16, tag="ln")

                # ---- mm1 + gates + logs (ACT fused over GF nd-tiles) ----
                GF = 5
                for g in range(NT_ALL // GF):
                    upg = upool.tile([128, GF * BJ], BF16, tag="upg")
                    for k in range(GF):
                        nt = g * GF + k
                        fv = ppa.tile([128, BJ], F32, tag="ps")
                        for ft in range(4):
                            nc.tensor.matmul(
                                fv[:],
                                wpt[:, bass.ts(nt * 4 + ft, 128)],
                                xsl(ft),
                                start=ft == 0, stop=ft == 3)
                        u1 = wpool.tile([128, BJ], F32, tag="u1")
                        nc.vector.tensor_scalar(u1[:], fv[:],
                                                c0t[:, nt:nt + 1],
                                                1.0, ALU.add, ALU.min)
                        nc.vector.tensor_scalar(upg[:, bass.ts(k, BJ)],
                                                u1[:], 0.0, None, ALU.max)
                    nc.scalar.activation(
                        lpt[:, g * GF * BJ:(g + 1) * GF * BJ], upg[:],
                        AF.Ln, bias=bze[:, 0:1], scale=1.0)
                    nlo = min(max(NT_LO - g * GF, 0), GF)   # lo tiles in group
                    if nlo:
                        nc.scalar.activation(
                            lnt[:, g * GF * BJ:(g * GF + nlo) * BJ],
                            upg[:, 0:nlo * BJ],
                            AF.Ln, bias=bze[:, 1:2], scale=-1.0)

                def lsrc(s, tau):
                    src = lpt if s == 1 else lnt
                    return src[:, bass.ts(tau, BJ)]

                # ---- per-om stream: path sums -> exp -> z -> n*z -> reduce
                for om in range(NOM):
                    ptb = kpool.tile([128, 4 * BJ], BF16, tag="ptb")
                    eag = []
                    for half in range(2):
                        sa2 = pps.tile([128, 2 * BJ], F32, tag="S")
                        for k in range(2):
                            passes = sa_passes[4 * om + 2 * half + k]
                            for i, (pi, tau, s) in enumerate(passes):
                                nc.tensor.matmul(
                                    sa2[:, bass.ts(k, BJ)],
                                    pbt[:, bass.ts(pi, 128)], lsrc(s, tau),
                                    start=i == 0, stop=i == len(passes) - 1)
                        ea = epool.tile([128, 2 * BJ], F32, tag="ea")
                        nc.scalar.activation(ea[:], sa2[:], AF.Exp)
                        eag.append(ea)
                    for jj in range(4):
                        al = 4 * om + jj
                        sb2 = pps.tile([128, 2 * BJ], F32, tag="S")
                        for k in range(2):
                            passes = sb_passes[2 * al + k]
                            for i, (pi, tau, s) in enumerate(passes):
                                nc.tensor.matmul(
                                    sb2[:, bass.ts(k, BJ)],
                                    pbt[:, bass.ts(pi, 128)], lsrc(s, tau),
                                    start=i == 0, stop=i == len(passes) - 1)
                        eb = epool.tile([128, 2 * BJ], BF16, tag="eb")
                        nc.scalar.activation(eb[:], sb2[:], AF.Exp)
                        z = ppz.tile([128, BJ], F32, tag="z")
                        for k in range(2):
                            nc.tensor.matmul(
                                z[:], r2t[:, bass.ts(2 * al + k, 128)],
                                eb[:, bass.ts(k, BJ)],
                                start=k == 0, stop=k == 1)
                        nc.vector.tensor_tensor(
                            ptb[:, bass.ts(jj, BJ)],
                            eag[jj // 2][:, bass.ts(jj % 2, BJ)],
                            z[:], ALU.mult)
                    outp = ppo.tile([128, BJ], F32, tag="o")
                    for jj in range(4):
                        nc.tensor.matmul(
                            outp[:], ont[:, bass.ts(jj, 128)],
                            ptb[:, bass.ts(jj, BJ)],
                            start=jj == 0, stop=jj == 3)
                    osb = opool.tile([128, BJ], BF16, tag="osb")
                    nc.vector.tensor_copy(osb[:], outp[:])
                    nc.sync.dma_start(
                        out_d[bass.ts(om, 128), BJ * j:BJ * (j + 1)], osb[:])

    if FIXUP_WAITS:
        _split_excess_waits(nc)
    return nc


def _split_excess_waits(nc):
    """Walrus codegen only fits ONE sync wait on PE Matmult and DMACopy
    instructions ("Too many sync wait commands").  Hoist the extras onto
    preceding same-engine InstEventSemaphore pseudos (one wait each), which
    the sequencer executes before the limited instruction."""
    exempt = {"InstEventSemaphore", "InstUnconditionalBranch",
              "InstISA", "InstHalt"}
    nfix = 0
    for fn in nc.m.functions:
        for bb in fn.blocks:
            il = bb.instructions
            out = []
            for inst in il:
                si = inst.sync_info
                lim = None if type(inst).__name__ in exempt else 1
                if si is not None and lim is not None and len(si.on_wait) > lim:
                    keep = list(si.on_wait[-lim:])
                    for w in si.on_wait[:-lim]:
                        nfix += 1
                        ev = mybir.InstEventSemaphore(
                            name=f"I-waitfix-{nfix}",
                            engine=inst.engine,
                            ins=[], outs=[],
                            sync_info=mybir.SyncInfo(on_wait=[w], on_update=[]),
                        )
                        ev.bass_nofuse = True
                        out.append(ev)
                    inst.sync_info = mybir.SyncInfo(
                        on_wait=keep, on_update=list(si.on_update))
                out.append(inst)
            il[:] = out
            assert len(bb.instructions) == len(out)
    return nfix


def _prep(fa, thr, lt, resp, path_map):
    h = hashlib.sha256()
    for a in (fa, thr, lt, resp, path_map):
        h.update(np.ascontiguousarray(a).tobytes())
    key = (h.hexdigest(), NCORES, BJ, REPEAT)
    if key not in _CACHE:
        consts = _host_consts(fa, thr, lt, resp, path_map)
        nc = _build_nc(consts)
        _CACHE[key] = nc
    return _CACHE[key]


def build_in_maps(x, feat_attention, feature_thresholds, log_temperatures,
                  response, path_map):
    nc = _prep(np.asarray(feat_attention, np.float32),
               np.asarray(feature_thresholds, np.float32),
               np.asarray(log_temperatures, np.float32),
               np.asarray(response, np.float32),
               np.asarray(path_map))
    BC = B // NCORES
    xf = np.asarray(x, np.float32)
    scale = np.abs(xf).max(axis=0) / 127.0                  # per feature [F]
    scale[scale == 0.0] = 1.0
    xq = np.clip(np.rint(xf / scale), -127, 127).astype(np.int8)
    xT = np.ascontiguousarray(xq.T)                          # [F, B] int8
    xs = np.ascontiguousarray(scale.reshape(4, 128).T.astype(np.float32))
    in_maps = [{"x": np.ascontiguousarray(xT[:, c * BC:(c + 1) * BC]),
                "xs": xs}
               for c in range(NCORES)]
    return in_maps, nc


def kernel(x, feat_attention, feature_thresholds, log_temperatures,
           response, path_map):
    in_maps, nc = build_in_maps(x, feat_attention, feature_thresholds,
                                log_temperatures, response, path_map)
    res = run_bass_kernel_spmd(nc, in_maps, list(range(NCORES)))
    global LAST
    LAST = res
    BC = B // NCORES
    out = np.empty((B, N), np.float32)
    for c in range(NCORES):
        out[c * BC:(c + 1) * BC, :] = \
            np.asarray(res.results[c]["out_t"], np.float32).T
    return out

